# revision 1
# baseline (speedup 1.0000x reference)
"""CSNN (spiking conv net with WTA dynamics) on 8 Trainium2 NeuronCores.

Key insight: the reference's "global" fire check `any(pot > threshold)` is
equivalent to a per-column check. After every fire the touched column is
softmax-reset to values < 1 <= threshold, so the global max only crosses the
threshold via the column touched by the current event. Hence every output
column evolves independently and the event scan vectorizes across columns:
columns ride the 128 SBUF partitions, output channels ride the free dim.

Per layer the host sorts each column's events by spike time (ties broken by
flat index, replicating the reference's stable argsort) and pre-gathers the
weight rows into a (P, S*F) stream; zero rows pad columns with fewer events
(they add 0 and can never fire, so they are exact no-ops). The device runs S
sequential steps; each step does: accumulate, max, softmax (exp on ScalarE
with per-partition bias, sum via accum_out), winner-zero via match_replace
(first-occurrence semantics == jnp.argmax tie-breaking), predicated commit,
and a max-accumulated spike-time record (valid because event times are
ascending). Layers are separate launches (the next layer's event order
depends on the previous layer's output); columns are sharded 8 ways.
"""
import numpy as np

import concourse.bacc as bacc
import concourse.mybir as mybir
from concourse.tile import TileContext
from concourse import bass_utils

F32 = np.float32
BF32 = mybir.dt.float32
SENT = -3.0e38
Exp = mybir.ActivationFunctionType.Exp
ALU = mybir.AluOpType

LAYERS = [
    dict(cout=30, k=5, pad=2, th=2.4),
    dict(cout=100, k=3, pad=1, th=1.0),
    dict(cout=200, k=3, pad=1, th=1.0),
]
N_CORES = 8


# ---------------------------------------------------------------- host side

def _unfold_buggy(x, k):
    C, H, W = x.shape
    oh, ow = H - k + 1, W - k + 1
    ih = np.arange(oh)[:, None] + np.arange(k)[None, :]
    iw = np.arange(ow)[:, None] + np.arange(k)[None, :]
    p = x[:, ih[:, None, :, None], iw[None, :, None, :]]
    unf = p.transpose(0, 3, 4, 1, 2).reshape(C * k * k, oh * ow)
    return unf.reshape(C, oh * ow, k * k), oh, ow


def _build_events(spk_in, weights, pad):
    cout, cin, k, _ = weights.shape
    x = np.pad(spk_in.astype(F32), ((0, 0), (pad, pad), (pad, pad)))
    x_trans, oh, ow = _unfold_buggy(x, k)
    L, k2 = oh * ow, k * k
    w_r = np.ascontiguousarray(weights.reshape(cout, cin * k2).T.astype(F32))
    tv = x_trans.transpose(1, 0, 2).reshape(L, cin * k2)
    order = np.argsort(np.where(tv != 0, tv, np.inf), axis=1, kind='stable')
    nvalid = (tv != 0).sum(axis=1)
    S = max(1, int(nvalid.max()))
    order = order[:, :S]
    tsort = np.take_along_axis(tv, order, axis=1)
    valid = np.arange(S)[None, :] < nvalid[:, None]
    W_seq = w_r[order]
    W_seq[~valid] = 0.0
    T_seq = np.where(valid, tsort, 0.0).astype(F32)
    return np.ascontiguousarray(W_seq), T_seq, S, oh, ow


def _shard(W_seq, T_seq):
    L, S, F = W_seq.shape
    Pc = (L + N_CORES - 1) // N_CORES
    Wp = np.zeros((Pc * N_CORES, S, F), F32)
    Tp = np.zeros((Pc * N_CORES, S), F32)
    Wp[:L] = W_seq
    Tp[:L] = T_seq
    Ws = [np.ascontiguousarray(Wp[i * Pc:(i + 1) * Pc].reshape(Pc, S * F))
          for i in range(N_CORES)]
    Ts = [np.ascontiguousarray(Tp[i * Pc:(i + 1) * Pc]) for i in range(N_CORES)]
    return Ws, Ts, Pc


def _max_pool2(x):
    C, H, W = x.shape
    oh, ow = H // 2, W // 2
    return x[:, :oh * 2, :ow * 2].reshape(C, oh, 2, ow, 2).max(axis=(2, 4))


# -------------------------------------------------------------- device side

def _build_layer(P, F, S, TH, CS=None):
    """One WTA layer: P columns on partitions, F channels on free dim,
    S time-ordered event steps."""
    if CS is None:
        CS = max(1, min(S, (48 * 1024) // (F * 4)))
    nc = bacc.Bacc("TRN2", target_bir_lowering=False, debug=False)
    Wd = nc.dram_tensor("W", (P, S * F), BF32, kind="ExternalInput")
    Td = nc.dram_tensor("T", (P, S), BF32, kind="ExternalInput")
    Od = nc.dram_tensor("spk", (P, F), BF32, kind="ExternalOutput")

    with TileContext(nc) as tc:
        with (
            tc.tile_pool(name="state", bufs=1) as st,
            tc.tile_pool(name="wpool", bufs=3) as wp,
        ):
            pot = st.tile([P, F], BF32)
            spk = st.tile([P, F], BF32)
            e = st.tile([P, F], BF32)
            e2 = st.tile([P, F], BF32)
            sm2 = st.tile([P, F], BF32)
            fireb = st.tile([P, F], mybir.dt.uint8)
            dd = st.tile([P, F], BF32)
            ones = st.tile([P, F], BF32)
            rz = st.tile([P, 8], BF32)   # col0 = e^m (match key), rest sentinel
            rt = st.tile([P, 1], BF32)   # r = 1/Z
            negm = st.tile([P, 1], BF32)
            Z = st.tile([P, 1], BF32)
            tg = st.tile([P, 1], BF32)
            Tt = st.tile([P, S], BF32)

            nc.vector.memset(pot[:], 0.0)
            nc.vector.memset(spk[:], 0.0)
            nc.vector.memset(ones[:], 1.0)
            nc.vector.memset(rz[:], SENT)
            nc.sync.dma_start(Tt[:], Td[:])

            for ci in range((S + CS - 1) // CS):
                s0, s1 = ci * CS, min(S, ci * CS + CS)
                wt = wp.tile([P, (s1 - s0) * F], BF32, tag="w")
                nc.sync.dma_start(wt[:], Wd[:, s0 * F:s1 * F])
                for s in range(s0, s1):
                    ws = wt[:, (s - s0) * F:(s - s0 + 1) * F]
                    nc.vector.tensor_add(pot[:], pot[:], ws)
                    nc.vector.tensor_reduce(
                        negm[:], pot[:], mybir.AxisListType.X, ALU.max, negate=True)
                    # unnormalized softmax: pot <= th+max_w < 3.5, exp safe.
                    # Does not wait on the max.
                    nc.scalar.activation(e[:], pot[:], Exp, accum_out=Z[:])
                    # match key e^m == e[winner] bit-exact (same spline, same input)
                    nc.scalar.activation(rz[:, 0:1], negm[:], Exp, scale=-1.0)
                    nc.vector.reciprocal(rt[:], Z[:])
                    nc.vector.match_replace(e2[:], rz[:], e[:], 0.0)
                    # normalize after winner-zero: fl(e_i*r) identical either way
                    nc.scalar.mul(sm2[:], e2[:], rt[:, 0:1])
                    nc.vector.tensor_scalar(fireb[:], ones[:], negm[:, 0:1], -TH,
                                            ALU.mult, ALU.is_lt)
                    # winner one-hot from pre-commit pot (top-2 margin >> rounding)
                    nc.vector.tensor_scalar(dd[:], pot[:], -1.0, negm[:, 0:1],
                                            ALU.mult, ALU.is_equal)
                    nc.vector.copy_predicated(pot[:], fireb[:], sm2[:])
                    nc.vector.scalar_tensor_tensor(tg[:], negm[:], -TH, Tt[:, s:s + 1],
                                                   ALU.is_lt, ALU.mult)
                    nc.vector.scalar_tensor_tensor(spk[:], dd[:], tg[:, 0:1], spk[:],
                                                   ALU.mult, ALU.max)
            nc.sync.dma_start(Od[:], spk[:])
    nc.finalize()
    return nc


_LAYER_RESULTS_NS = []


def _run_layer(Ws, Ts, F, TH, S, Pc, trace=False):
    nc = _build_layer(Pc, F, S, TH)
    in_maps = [{"W": w, "T": t} for w, t in zip(Ws, Ts)]
    res = bass_utils.run_bass_kernel_spmd(
        nc, in_maps, core_ids=list(range(N_CORES)), trace=trace)
    _LAYER_RESULTS_NS.append(res.exec_time_ns)
    return [r["spk"] for r in res.results]


def kernel(x, w1, w2, w3, _trace=False):
    _LAYER_RESULTS_NS.clear()
    s = np.asarray(x, F32)
    for w, cfg in zip((w1, w2, w3), LAYERS):
        W_seq, T_seq, S, oh, ow = _build_events(s, np.asarray(w, F32), cfg['pad'])
        Ws, Ts, Pc = _shard(W_seq, T_seq)
        spks = _run_layer(Ws, Ts, cfg['cout'], cfg['th'], S, Pc, trace=_trace)
        full = np.concatenate(spks, axis=0)[:oh * ow]
        s = _max_pool2(np.ascontiguousarray(full.T.reshape(cfg['cout'], oh, ow)))
    return np.ascontiguousarray(s)

